# revision 14
# baseline (speedup 1.0000x reference)
"""Trainium2 Bass kernel for dilated 5x7 conv (128->16ch) + 1x1 (16->16) + 1x1 (16->128).

Strategy (data-parallel, 1 image per core across 8 cores):
  reference: y = conv_dilated(x, w3, dil=(2,3), pad=(4,9)); y = w4@y; y = w5@y
  Host folds w45 = w5 @ w4  [128, 16].

  Per core, image x [128, 56, 56] zero-padded to xp [128, 64, 74] (bf16):
  Stage 1 (TensorE): for each kw in 0..6, one matmul with
      lhsT = w1[:, kw, :] [c=128, (kh,co)=80], rhs = xp[:, rows, 3kw:3kw+56],
      PSUM-accumulating over kw  ->  P2[(kh,co), r, w] =
      sum_{kw,c} w3[co,c,kh,kw] * xp[c, r, w+3kw].
  Evacuate PSUM->SBUF with f32->bf16 cast (ScalarE).
  Shift-align (DMA, free-dim row offsets on same partitions):
      P2a[(kh,co), h, w] = P2[(kh,co), h+2kh, w]   for h in 0..55.
  Stage 2 (TensorE): out[o, h, w] = sum_{(kh,co)} w2[(kh,co), o] * P2a[(kh,co), h, w]
      -- a single K=80, M=128 matmul per 8-row chunk, w2[(kh,co), o] = w45[o, co].
  Evacuate (VectorE) and DMA out (f32).
"""

import os
import sys

import numpy as np

for _p in ("/opt/trn_rl_repo", "/root/.axon_site/_ro/trn_rl_repo"):
    if os.path.isdir(_p) and _p not in sys.path:
        sys.path.insert(0, _p)

import ml_dtypes  # noqa: E402

import concourse.bass as bass  # noqa: E402
import concourse.tile as tile  # noqa: E402
from concourse.tile_rust import add_dep_helper  # noqa: E402
from concourse import mybir  # noqa: E402
from concourse.bass_utils import run_bass_kernel_spmd  # noqa: E402

N, C, H, W = 8, 128, 56, 56
CO = 16
KH, KW = 5, 7
DH, DW = 2, 3
PH, PW = 4, 9
RP, WP = H + 2 * PH, W + 2 * PW  # 64 padded rows, 74 padded cols
M1 = KH * CO  # 80
RCH = 8  # stage-1 chunk: input rows per chunk
NCH1 = RP // RCH  # 8
OCH = 8  # stage-2 chunk: output rows per chunk
NCH2 = H // OCH  # 7
BF16 = mybir.dt.bfloat16
F32 = mybir.dt.float32

_NC = None


def _build_nc(attempt=0):
    nc = bass.Bass(
        "TRN2",
        target_bir_lowering=False,
        debug=False,
        enable_asserts=False,
        num_devices=N,
    )
    # all weights in ONE dram tensor/DMA so a single dummy matmul can absorb
    # the weight-DMA wait (the MM ISA slot fits only one semaphore wait).
    WKC = KW * M1 + KH * C  # 560 + 640
    xp_d = nc.dram_tensor("xp", [C, RP, WP], BF16, kind="ExternalInput")
    wk_d = nc.dram_tensor("wk", [C, WKC], BF16, kind="ExternalInput")
    out_d = nc.dram_tensor("out", [C, H * W], F32, kind="ExternalOutput")

    with tile.TileContext(nc) as tc:
        # schedule perturbation for compile-retry: the Tile scheduler is
        # process-state dependent and occasionally emits a tail Drain with
        # more semaphore waits than the ISA slot fits; a few extra leading
        # nops reshuffle the schedule.
        for _ in range(attempt):
            nc.sync.nop(nofuse=True)
        with (
            tc.tile_pool(name="const", bufs=1) as constp,
            tc.tile_pool(name="xin", bufs=1) as xinp,
            tc.tile_pool(name="p2s", bufs=1) as p2sp,
            tc.tile_pool(name="outs", bufs=1) as outsp,
            tc.tile_pool(name="psd", bufs=1, space="PSUM") as psd,
            tc.tile_pool(name="ps1", bufs=3, space="PSUM") as ps1,
            tc.tile_pool(name="ps2", bufs=4, space="PSUM") as ps2,
        ):
            in_dmas = []
            wk_t = constp.tile([C, WKC], BF16, tag="wk")
            in_dmas.append(nc.sync.dma_start(wk_t[:], wk_d.ap()))
            w1_t = wk_t[:, 0 : KW * M1].rearrange("c (kw m) -> c kw m", kw=KW)
            w2_t = wk_t[0:M1, KW * M1 :].rearrange("p (kh o) -> p kh o", kh=KH)

            xp_t = xinp.tile([C, RP, WP], BF16, tag="xp")
            in_dmas.append(nc.sync.dma_start(xp_t[:], xp_d.ap()))

            p2s_t = p2sp.tile([M1, RP, W], BF16)
            outsb_t = outsp.tile([C, H * W], F32)
            out_ap = out_d.ap()
            # 3 coarse out-DMAs (6 DMAs total <= 8 HWDGE queues, so no
            # same-queue WAW wait lands on any single-wait-slot DMA).
            dma_cuts = {2: (0, 3), 4: (3, 5), 6: (5, 7)}
            out_dmas = []

            # dummy matmul: first PE instruction, absorbs the wk-DMA wait so
            # every later matmul needs at most one new semaphore wait.
            dt = psd.tile([1, 1], F32, tag="dummy")
            nc.tensor.matmul(dt[:], wk_t[0:M1, 0:1], wk_t[0:M1, 0:1], start=True, stop=True)

            for k in range(NCH1):
                xt = xp_t[:, k * RCH : (k + 1) * RCH, :]
                pt = ps1.tile([M1, RCH, W], F32, tag="p1")
                for kw in range(KW):
                    nc.tensor.matmul(
                        pt[:],
                        w1_t[:, kw, :],
                        xt[:, :, DW * kw : DW * kw + W],
                        start=(kw == 0),
                        stop=(kw == KW - 1),
                    )
                nc.scalar.copy(p2s_t[:, k * RCH : (k + 1) * RCH, :], pt[:])

            last_mm = None
            last_cp = None
            for j in range(NCH2):
                qt = ps2.tile([C, OCH, W], F32, tag="p2")
                for kh in range(KH):
                    r0 = j * OCH + DH * kh
                    last_mm = nc.tensor.matmul(
                        qt[:],
                        w2_t[:, kh, :],
                        p2s_t[:, r0 : r0 + OCH, :],
                        start=(kh == 0),
                        stop=(kh == KH - 1),
                    )
                last_cp = nc.scalar.copy(
                    outsb_t[:, j * OCH * W : (j + 1) * OCH * W], qt[:]
                )
                if j in dma_cuts:
                    a, b = dma_cuts[j]
                    out_dmas.append(
                        nc.sync.dma_start(
                            out_ap[:, a * OCH * W : b * OCH * W],
                            outsb_t[:, a * OCH * W : b * OCH * W],
                        )
                    )

            # absorb each out-DMA completion into a chained SP nop so the
            # kernel-tail Drain (one wait slot per proc, few slots) only
            # needs engine semaphores, not per-DMA-queue ones.
            # absorb every proc's final tick into SP program order so the tail
            # Drain needs no (or one) semaphore wait in any schedule.
            for dep in in_dmas + out_dmas + [last_mm, last_cp]:
                nop = nc.sync.nop(nofuse=True)
                add_dep_helper(nop.ins, dep.ins, sync=True, reason="absorb tick")
    return nc


def _get_nc():
    global _NC
    if _NC is None:
        _NC = _build_nc()
    return _NC


def _prep_inputs(x, w3, w4, w5):
    w45 = (w5.astype(np.float64) @ w4.astype(np.float64)).astype(np.float32)
    # w1[c, kw, kh*CO+co] = w3[co, c, kh, kw]
    w1 = (
        np.transpose(w3, (1, 3, 2, 0))
        .reshape(C, KW, KH * CO)
        .astype(ml_dtypes.bfloat16)
    )
    # w2[p, kh, o] = w45[o, co] if p == kh*CO+co else 0  (zero rows kill the
    # blocks of p2s that belong to other kh taps in the K=80 contraction)
    w2 = np.zeros((M1, KH, C), np.float32)
    for kh in range(KH):
        w2[kh * CO : (kh + 1) * CO, kh, :] = w45.T
    wk = np.zeros((C, KW * M1 + KH * C), np.float32)
    wk[:, : KW * M1] = np.asarray(w1, np.float32).reshape(C, KW * M1)
    wk[:M1, KW * M1 :] = w2.reshape(M1, KH * C)
    wk = wk.astype(ml_dtypes.bfloat16)
    xp = np.zeros((N, C, RP, WP), np.float32)
    xp[:, :, PH : PH + H, PW : PW + W] = x
    xp = xp.astype(ml_dtypes.bfloat16)
    return xp, wk


def kernel(x, w3, w4, w5, trace=False):
    x = np.asarray(x, np.float32)
    w3 = np.asarray(w3, np.float32)
    w4 = np.asarray(w4, np.float32)
    w5 = np.asarray(w5, np.float32)
    xp, wk = _prep_inputs(x, w3, w4, w5)
    in_maps = [
        {"xp": np.ascontiguousarray(xp[n]), "wk": wk} for n in range(N)
    ]
    global _NC
    res = None
    last_err = None
    for attempt in range(6):
        if _NC is None:
            _NC = _build_nc(attempt)
        try:
            res = run_bass_kernel_spmd(
                _NC, in_maps, core_ids=list(range(N)), trace=trace
            )
            break
        except Exception as e:  # compile-schedule flake: rebuild perturbed
            last_err = e
            _NC = None
    if res is None:
        raise last_err
    out = np.stack(
        [np.asarray(res.results[n]["out"]).reshape(C, H, W) for n in range(N)]
    ).astype(np.float32)
    if trace:
        return out, res
    return out
